# revision 8
# baseline (speedup 1.0000x reference)
"""ConcatCritic MLP over the B^2 pair grid, Trainium2 Bass/Tile kernel.

out[i, j] = softplus(f(x[i], y[j])) where f is a 4-layer MLP on
concat(x, y). Reference pair grid: pairs[a*n+b] = concat(x[b], y[a]),
scores.reshape(n,n).T -> out.

Key factorization: layer 1 is linear in the concat, so
  h1[a,b] = relu(x[b] @ W1top + y[a] @ W1bot + b1)
with W1top = W1[:128], W1bot = W1[128:]. The [B^2, 256] layer-1 matmul
collapses into two tiny matmuls plus a per-partition broadcast add.

Layout: activations kept transposed [features, batch] so every layer's
matmul (weights stationary as lhsT [K, M]) writes the next layer's rhs
directly: out[m=feat, n=j] = sum_k W[k, m] * hT[k, j].

Sharding: core c owns y rows [c*64, (c+1)*64); computes block
V_c[il, j] = f(x[j], y[c*64+il]) of shape [64, 512]. Host gathers
V = concat(V_c) and returns V.T.

Self-contained: hardcodes shapes; imports concourse from the system repo.
"""

import os
import sys

import numpy as np


def _import_concourse():
    try:
        import concourse  # noqa: F401
        return
    except ImportError:
        pass
    for p in ("/opt/trn_rl_repo", "/root/.axon_site/_ro/trn_rl_repo"):
        if os.path.isdir(p) and p not in sys.path:
            sys.path.insert(0, p)
    import concourse  # noqa: F401


_import_concourse()

import concourse.bacc as bacc  # noqa: E402
import concourse.tile as tile  # noqa: E402
from concourse import mybir  # noqa: E402
from concourse.bass_utils import run_bass_kernel_spmd  # noqa: E402

B = 512          # batch (pair-grid side)
D = 128          # input dim per tensor
H = 256          # hidden dim
NCORES = 8
RB = B // NCORES  # 64 y-rows per core
F32 = mybir.dt.float32

# float32r: fp32 bits in memory, single-pass reduced-precision multiply on
# the PE at 1 cycle/row (vs 4 for full fp32). Flip to mybir.dt.float32 for
# bit-accurate matmuls.
MM_DT = mybir.dt.float32


def _mm(ap):
    """Bitcast an fp32 AP to the matmul dtype (no-op for float32)."""
    if MM_DT == F32:
        return ap
    return ap.bitcast(MM_DT)


def _emit(tc, nc, d, out_d):
    AF = mybir.ActivationFunctionType
    OP = mybir.AluOpType
    from contextlib import ExitStack

    with ExitStack() as ctx:
        const = ctx.enter_context(tc.tile_pool(name="const", bufs=1))
        hpool = ctx.enter_context(tc.tile_pool(name="h", bufs=2))
        psum = ctx.enter_context(tc.tile_pool(name="psum", bufs=1, space="PSUM"))

        def load(name, shape, src_ap=None):
            t = const.tile(list(shape), F32, tag=name, name=name + "_s")
            nc.sync.dma_start(out=t[:], in_=(src_ap if src_ap is not None else d[name][:]))
            return t

        xT = load("xT", (D, B))
        yT = load("yT", (D, RB))
        w1t = load("W1t", (D, H))
        w1b = load("W1b", (D, H))
        w2 = [load(f"W2_{k}", (128, H), d["W2"][k * 128:(k + 1) * 128, :]) for k in range(2)]
        w3 = [load(f"W3_{k}", (128, H), d["W3"][k * 128:(k + 1) * 128, :]) for k in range(2)]
        w4 = [load(f"W4_{k}", (128, 1), d["W4"][k * 128:(k + 1) * 128, :]) for k in range(2)]
        b1c = [load(f"b1_{k}", (128, 1), d["b1"][k * 128:(k + 1) * 128, :]) for k in range(2)]
        b2c = [load(f"b2_{k}", (128, 1), d["b2"][k * 128:(k + 1) * 128, :]) for k in range(2)]
        b3c = [load(f"b3_{k}", (128, 1), d["b3"][k * 128:(k + 1) * 128, :]) for k in range(2)]
        b4r = load("b4r", (RB, 1))

        # ---- preamble: xa[oc] = (x @ W1top)^T + b1 (bias folded here),
        #                yb[oc] = (y_slice @ W1bot)^T
        xa = []
        yb = []
        for oc in range(2):
            ms = slice(oc * 128, (oc + 1) * 128)
            pxa = psum.tile([128, B], F32, tag="pre", name=f"pxa{oc}")
            nc.tensor.matmul(pxa[:], lhsT=w1t[:, ms], rhs=xT[:], start=True, stop=True)
            xat = const.tile([128, B], F32, tag=f"xa{oc}", name=f"xa{oc}")
            nc.vector.tensor_scalar(xat[:], pxa[:], b1c[oc][:, 0:1], None, OP.add)
            xa.append(xat)
        for oc in range(2):
            ms = slice(oc * 128, (oc + 1) * 128)
            pyb = psum.tile([128, RB], F32, tag="pre", name=f"pyb{oc}")
            nc.tensor.matmul(pyb[:], lhsT=w1b[:, ms], rhs=yT[:], start=True, stop=True)
            ybt = const.tile([128, RB], F32, tag=f"yb{oc}", name=f"yb{oc}")
            nc.vector.tensor_copy(ybt[:], pyb[:])
            yb.append(ybt)

        score = const.tile([RB, B], F32, tag="score", name="score")

        for i in range(RB):
            # ---- layer 1: h1[oc] = relu(xa[oc] + yb[oc][:, i])  (DVE)
            h1 = []
            for oc in range(2):
                t = hpool.tile([128, B], F32, tag=f"h1_{oc}", name=f"h1_{oc}_{i}")
                nc.vector.tensor_scalar(
                    t[:], xa[oc][:], yb[oc][:, i:i + 1], 0.0, OP.add, OP.max
                )
                h1.append(t)

            # ---- layer 2: h2[mc] = relu(W2[:, mc].T @ h1 + b2[mc])
            h2 = []
            for mc in range(2):
                ms = slice(mc * 128, (mc + 1) * 128)
                p = psum.tile([128, B], F32, tag=f"l2_{mc}", name=f"p2_{mc}_{i}")
                for kc in range(2):
                    nc.tensor.matmul(
                        p[:], lhsT=_mm(w2[kc][:, ms]), rhs=_mm(h1[kc][:]),
                        start=(kc == 0), stop=(kc == 1),
                    )
                t = hpool.tile([128, B], F32, tag=f"h2_{mc}", name=f"h2_{mc}_{i}")
                nc.scalar.activation(t[:], p[:], AF.Relu, bias=b2c[mc][:, 0:1])
                h2.append(t)

            # ---- layer 3
            h3 = []
            for mc in range(2):
                ms = slice(mc * 128, (mc + 1) * 128)
                p = psum.tile([128, B], F32, tag=f"l3_{mc}", name=f"p3_{mc}_{i}")
                for kc in range(2):
                    nc.tensor.matmul(
                        p[:], lhsT=_mm(w3[kc][:, ms]), rhs=_mm(h2[kc][:]),
                        start=(kc == 0), stop=(kc == 1),
                    )
                t = hpool.tile([128, B], F32, tag=f"h3_{mc}", name=f"h3_{mc}_{i}")
                nc.scalar.activation(t[:], p[:], AF.Relu, bias=b3c[mc][:, 0:1])
                h3.append(t)

            # ---- layer 4: score[i, :] = W4.T @ h3  (bias b4 folded into tail).
            # Engines can only write partition bases {0,32,64,96}, so the
            # [1, B] result is staged through a base-0 tile and DMA'd into
            # row i of the score tile (DMA can target any partition).
            p4 = psum.tile([1, B], F32, tag="l4", name=f"p4_{i}", bufs=2)
            for kc in range(2):
                nc.tensor.matmul(
                    p4[:], lhsT=_mm(w4[kc][:]), rhs=_mm(h3[kc][:]),
                    start=(kc == 0), stop=(kc == 1),
                )
            s4 = hpool.tile([1, B], F32, tag="s4", name=f"s4_{i}")
            nc.vector.tensor_copy(s4[:], p4[:])
            nc.sync.dma_start(out=score[i:i + 1, :], in_=s4[:])

        # ---- tail: softplus(score + b4) = ln(1 + exp(score + b4)), batched
        # over all 64 rows so the ACT ops run with full partition parallelism.
        # Relu/Exp/Ln all live in the natural_log_exp_and_others ACT table, so
        # the whole kernel needs a single table load.
        e_t = const.tile([RB, B], F32, tag="e_t", name="e_t")
        nc.scalar.activation(e_t[:], score[:], AF.Exp, bias=b4r[:, 0:1])
        fin = const.tile([RB, B], F32, tag="fin", name="fin")
        nc.scalar.activation(fin[:], e_t[:], AF.Ln, bias=1.0)

        nc.sync.dma_start(out=out_d[:], in_=fin[:])


def _build_program():
    nc = bacc.Bacc("TRN2", target_bir_lowering=False, debug=False, enable_asserts=False)
    d = {}
    for name, shape in [
        ("xT", (D, B)), ("yT", (D, RB)),
        ("W1t", (D, H)), ("W1b", (D, H)),
        ("W2", (H, H)), ("W3", (H, H)), ("W4", (H, 1)),
        ("b1", (H, 1)), ("b2", (H, 1)), ("b3", (H, 1)), ("b4r", (RB, 1)),
    ]:
        d[name] = nc.dram_tensor(name, list(shape), F32, kind="ExternalInput").ap()
    out_d = nc.dram_tensor("out", [RB, B], F32, kind="ExternalOutput").ap()
    with tile.TileContext(nc) as tc:
        _emit(tc, nc, d, out_d)
    nc.compile()
    return nc


_PROGRAM = None


def _get_program():
    global _PROGRAM
    if _PROGRAM is None:
        _PROGRAM = _build_program()
    return _PROGRAM


def _make_in_maps(x, y, W1, b1, W2, b2, W3, b3, W4, b4):
    f = np.float32
    xT = np.ascontiguousarray(x.T, dtype=f)
    shared = {
        "xT": xT,
        "W1t": np.ascontiguousarray(W1[:D], dtype=f),
        "W1b": np.ascontiguousarray(W1[D:], dtype=f),
        "W2": np.ascontiguousarray(W2, dtype=f),
        "W3": np.ascontiguousarray(W3, dtype=f),
        "W4": np.ascontiguousarray(W4.reshape(H, 1), dtype=f),
        "b1": np.ascontiguousarray(b1.reshape(H, 1), dtype=f),
        "b2": np.ascontiguousarray(b2.reshape(H, 1), dtype=f),
        "b3": np.ascontiguousarray(b3.reshape(H, 1), dtype=f),
        "b4r": np.full((RB, 1), np.asarray(b4, dtype=f).reshape(-1)[0], dtype=f),
    }
    in_maps = []
    for c in range(NCORES):
        m = dict(shared)
        m["yT"] = np.ascontiguousarray(y[c * RB:(c + 1) * RB].T, dtype=f)
        in_maps.append(m)
    return in_maps


def _run(inputs, trace=False, trace_cores=None):
    nc = _get_program()
    in_maps = _make_in_maps(**inputs)
    res = run_bass_kernel_spmd(
        nc, in_maps, list(range(NCORES)), trace=trace, trace_cores=trace_cores,
    )
    V = np.concatenate([res.results[c]["out"] for c in range(NCORES)], axis=0)
    out = np.ascontiguousarray(V.T, dtype=np.float32)
    return out, res


def kernel(**inputs):
    out, _ = _run(inputs, trace=False)
    return out


# revision 12
# speedup vs baseline: 2.5932x; 2.5932x over previous
"""ConcatCritic MLP over the B^2 pair grid, Trainium2 Bass/Tile kernel.

out[i, j] = softplus(f(x[i], y[j])) where f is a 4-layer MLP on
concat(x, y). Reference pair grid: pairs[a*n+b] = concat(x[b], y[a]),
scores.reshape(n,n).T -> out.

Key factorization: layer 1 is linear in the concat, so
  h1[a,b] = relu(x[b] @ W1top + y[a] @ W1bot + b1)
with W1top = W1[:128], W1bot = W1[128:]. The [B^2, 256] layer-1 matmul
collapses into two tiny matmuls plus a per-partition broadcast add.

Layout: activations kept transposed [features, batch] so every layer's
matmul (weights stationary as lhsT [K, M]) writes the next layer's rhs
directly: out[m=feat, n=j] = sum_k W[k, m] * hT[k, j].

Sharding: core c owns y rows [c*64, (c+1)*64); computes block
V_c[il, j] = f(x[j], y[c*64+il]) of shape [64, 512]. Host gathers
V = concat(V_c) and returns V.T.

Self-contained: hardcodes shapes; imports concourse from the system repo.
"""

import os
import sys

import numpy as np


def _import_concourse():
    try:
        import concourse  # noqa: F401
        return
    except ImportError:
        pass
    for p in ("/opt/trn_rl_repo", "/root/.axon_site/_ro/trn_rl_repo"):
        if os.path.isdir(p) and p not in sys.path:
            sys.path.insert(0, p)
    import concourse  # noqa: F401


_import_concourse()

import concourse.bacc as bacc  # noqa: E402
import concourse.tile as tile  # noqa: E402
from concourse import mybir  # noqa: E402
from concourse.bass_utils import run_bass_kernel_spmd  # noqa: E402

B = 512          # batch (pair-grid side)
D = 128          # input dim per tensor
H = 256          # hidden dim
NCORES = 8
RB = B // NCORES  # 64 y-rows per core
F32 = mybir.dt.float32

# float32r: fp32 bits in memory, single-pass reduced-precision multiply on
# the PE at 1 cycle/row (vs 4 for full fp32). Flip to mybir.dt.float32 for
# bit-accurate matmuls.
MM_DT = mybir.dt.float32r


def _src(ap):
    """Bitcast a DRAM fp32 AP to the matmul dtype for DMA into typed tiles.

    Walrus requires every tensor consumed by an FP32r matmul to be produced
    as float32r (DMA passthrough of fp32 bits is fine — same bit layout),
    so weight/activation tiles are allocated with dtype MM_DT and their
    producers write that dtype directly.
    """
    if MM_DT == F32:
        return ap
    return ap.bitcast(MM_DT)


def _emit(tc, nc, d, out_d):
    AF = mybir.ActivationFunctionType
    OP = mybir.AluOpType
    from contextlib import ExitStack

    with ExitStack() as ctx:
        const = ctx.enter_context(tc.tile_pool(name="const", bufs=1))
        hpool = ctx.enter_context(tc.tile_pool(name="h", bufs=2))
        psum = ctx.enter_context(tc.tile_pool(name="psum", bufs=1, space="PSUM"))

        def load(name, shape, src_ap=None, dt=F32):
            t = const.tile(list(shape), dt, tag=name, name=name + "_s")
            src = src_ap if src_ap is not None else d[name][:]
            if dt != F32:
                src = src.bitcast(dt)
            nc.sync.dma_start(out=t[:], in_=src)
            return t

        xT = load("xT", (D, B))
        yT = load("yT", (D, RB))
        w1t = load("W1t", (D, H))
        w1b = load("W1b", (D, H))
        w2 = [load(f"W2_{k}", (128, H), d["W2"][k * 128:(k + 1) * 128, :], MM_DT) for k in range(2)]
        w3 = [load(f"W3_{k}", (128, H), d["W3"][k * 128:(k + 1) * 128, :], MM_DT) for k in range(2)]
        w4 = [load(f"W4_{k}", (128, 1), d["W4"][k * 128:(k + 1) * 128, :], MM_DT) for k in range(2)]
        b1c = [load(f"b1_{k}", (128, 1), d["b1"][k * 128:(k + 1) * 128, :]) for k in range(2)]
        b2c = [load(f"b2_{k}", (128, 1), d["b2"][k * 128:(k + 1) * 128, :]) for k in range(2)]
        b3c = [load(f"b3_{k}", (128, 1), d["b3"][k * 128:(k + 1) * 128, :]) for k in range(2)]
        b4r = load("b4r", (RB, 1))

        # ---- preamble: xa[oc] = (x @ W1top)^T + b1 (bias folded here),
        #                yb[oc] = (y_slice @ W1bot)^T
        xa = []
        yb = []
        for oc in range(2):
            ms = slice(oc * 128, (oc + 1) * 128)
            pxa = psum.tile([128, B], F32, tag="pre", name=f"pxa{oc}")
            nc.tensor.matmul(pxa[:], lhsT=w1t[:, ms], rhs=xT[:], start=True, stop=True)
            xat = const.tile([128, B], F32, tag=f"xa{oc}", name=f"xa{oc}")
            nc.vector.tensor_scalar(xat[:], pxa[:], b1c[oc][:, 0:1], None, OP.add)
            xa.append(xat)
        for oc in range(2):
            ms = slice(oc * 128, (oc + 1) * 128)
            pyb = psum.tile([128, RB], F32, tag="pre", name=f"pyb{oc}")
            nc.tensor.matmul(pyb[:], lhsT=w1b[:, ms], rhs=yT[:], start=True, stop=True)
            ybt = const.tile([128, RB], F32, tag=f"yb{oc}", name=f"yb{oc}")
            nc.vector.tensor_copy(ybt[:], pyb[:])
            yb.append(ybt)

        score = const.tile([RB, B], F32, tag="score", name="score")

        for i in range(RB):
            # ---- layer 1: h1[oc] = relu(xa[oc] + yb[oc][:, i])  (DVE)
            h1 = []
            for oc in range(2):
                t = hpool.tile([128, B], MM_DT, tag=f"h1_{oc}", name=f"h1_{oc}_{i}")
                nc.vector.tensor_scalar(
                    t[:], xa[oc][:], yb[oc][:, i:i + 1], 0.0, OP.add, OP.max
                )
                h1.append(t)

            # ---- layer 2: h2[mc] = relu(W2[:, mc].T @ h1 + b2[mc])
            h2 = []
            for mc in range(2):
                ms = slice(mc * 128, (mc + 1) * 128)
                p = psum.tile([128, B], F32, tag=f"l2_{mc}", name=f"p2_{mc}_{i}")
                for kc in range(2):
                    nc.tensor.matmul(
                        p[:], lhsT=w2[kc][:, ms], rhs=h1[kc][:],
                        start=(kc == 0), stop=(kc == 1),
                    )
                t = hpool.tile([128, B], MM_DT, tag=f"h2_{mc}", name=f"h2_{mc}_{i}")
                nc.scalar.activation(t[:], p[:], AF.Relu, bias=b2c[mc][:, 0:1])
                h2.append(t)

            # ---- layer 3
            h3 = []
            for mc in range(2):
                ms = slice(mc * 128, (mc + 1) * 128)
                p = psum.tile([128, B], F32, tag=f"l3_{mc}", name=f"p3_{mc}_{i}")
                for kc in range(2):
                    nc.tensor.matmul(
                        p[:], lhsT=w3[kc][:, ms], rhs=h2[kc][:],
                        start=(kc == 0), stop=(kc == 1),
                    )
                t = hpool.tile([128, B], MM_DT, tag=f"h3_{mc}", name=f"h3_{mc}_{i}")
                nc.scalar.activation(t[:], p[:], AF.Relu, bias=b3c[mc][:, 0:1])
                h3.append(t)

            # ---- layer 4: score[i, :] = W4.T @ h3  (bias b4 folded into tail).
            # Engines can only write partition bases {0,32,64,96}, so the
            # [1, B] result is staged through a base-0 tile and DMA'd into
            # row i of the score tile (DMA can target any partition).
            p4 = psum.tile([1, B], F32, tag="l4", name=f"p4_{i}", bufs=2)
            for kc in range(2):
                nc.tensor.matmul(
                    p4[:], lhsT=w4[kc][:], rhs=h3[kc][:],
                    start=(kc == 0), stop=(kc == 1),
                )
            s4 = hpool.tile([1, B], F32, tag="s4", name=f"s4_{i}")
            nc.vector.tensor_copy(s4[:], p4[:])
            nc.sync.dma_start(out=score[i:i + 1, :], in_=s4[:])

        # ---- tail: softplus(score + b4) = ln(1 + exp(score + b4)), batched
        # over all 64 rows so the ACT ops run with full partition parallelism.
        # Relu/Exp/Ln all live in the natural_log_exp_and_others ACT table, so
        # the whole kernel needs a single table load.
        e_t = const.tile([RB, B], F32, tag="e_t", name="e_t")
        nc.scalar.activation(e_t[:], score[:], AF.Exp, bias=b4r[:, 0:1])
        fin = const.tile([RB, B], F32, tag="fin", name="fin")
        nc.scalar.activation(fin[:], e_t[:], AF.Ln, bias=1.0)

        nc.sync.dma_start(out=out_d[:], in_=fin[:])


def _build_program():
    nc = bacc.Bacc("TRN2", target_bir_lowering=False, debug=False, enable_asserts=False)
    d = {}
    for name, shape in [
        ("xT", (D, B)), ("yT", (D, RB)),
        ("W1t", (D, H)), ("W1b", (D, H)),
        ("W2", (H, H)), ("W3", (H, H)), ("W4", (H, 1)),
        ("b1", (H, 1)), ("b2", (H, 1)), ("b3", (H, 1)), ("b4r", (RB, 1)),
    ]:
        d[name] = nc.dram_tensor(name, list(shape), F32, kind="ExternalInput").ap()
    out_d = nc.dram_tensor("out", [RB, B], F32, kind="ExternalOutput").ap()
    with tile.TileContext(nc) as tc:
        _emit(tc, nc, d, out_d)
    nc.compile()
    return nc


_PROGRAM = None


def _get_program():
    global _PROGRAM
    if _PROGRAM is None:
        _PROGRAM = _build_program()
    return _PROGRAM


def _make_in_maps(x, y, W1, b1, W2, b2, W3, b3, W4, b4):
    f = np.float32
    xT = np.ascontiguousarray(x.T, dtype=f)
    shared = {
        "xT": xT,
        "W1t": np.ascontiguousarray(W1[:D], dtype=f),
        "W1b": np.ascontiguousarray(W1[D:], dtype=f),
        "W2": np.ascontiguousarray(W2, dtype=f),
        "W3": np.ascontiguousarray(W3, dtype=f),
        "W4": np.ascontiguousarray(W4.reshape(H, 1), dtype=f),
        "b1": np.ascontiguousarray(b1.reshape(H, 1), dtype=f),
        "b2": np.ascontiguousarray(b2.reshape(H, 1), dtype=f),
        "b3": np.ascontiguousarray(b3.reshape(H, 1), dtype=f),
        "b4r": np.full((RB, 1), np.asarray(b4, dtype=f).reshape(-1)[0], dtype=f),
    }
    in_maps = []
    for c in range(NCORES):
        m = dict(shared)
        m["yT"] = np.ascontiguousarray(y[c * RB:(c + 1) * RB].T, dtype=f)
        in_maps.append(m)
    return in_maps


def _run(inputs, trace=False, trace_cores=None):
    nc = _get_program()
    in_maps = _make_in_maps(**inputs)
    res = run_bass_kernel_spmd(
        nc, in_maps, list(range(NCORES)), trace=trace, trace_cores=trace_cores,
    )
    V = np.concatenate([res.results[c]["out"] for c in range(NCORES)], axis=0)
    out = np.ascontiguousarray(V.T, dtype=np.float32)
    return out, res


def kernel(**inputs):
    out, _ = _run(inputs, trace=False)
    return out


# revision 14
# speedup vs baseline: 3.4699x; 1.3381x over previous
"""ConcatCritic MLP over the B^2 pair grid, Trainium2 Bass/Tile kernel.

out[i, j] = softplus(f(x[i], y[j])) where f is a 4-layer MLP on
concat(x, y). Reference pair grid: pairs[a*n+b] = concat(x[b], y[a]),
scores.reshape(n,n).T -> out.

Key factorization: layer 1 is linear in the concat, so
  h1[a,b] = relu(x[b] @ W1top + y[a] @ W1bot + b1)
with W1top = W1[:128], W1bot = W1[128:]. The [B^2, 256] layer-1 matmul
collapses into two tiny matmuls plus a per-partition broadcast add.

Layout: activations kept transposed [features, batch] so every layer's
matmul (weights stationary as lhsT [K, M]) writes the next layer's rhs
directly: out[m=feat, n=j] = sum_k W[k, m] * hT[k, j].

Sharding: core c owns y rows [c*64, (c+1)*64); computes block
V_c[il, j] = f(x[j], y[c*64+il]) of shape [64, 512]. Host gathers
V = concat(V_c) and returns V.T.

Self-contained: hardcodes shapes; imports concourse from the system repo.
"""

import os
import sys

import numpy as np


def _import_concourse():
    try:
        import concourse  # noqa: F401
        return
    except ImportError:
        pass
    for p in ("/opt/trn_rl_repo", "/root/.axon_site/_ro/trn_rl_repo"):
        if os.path.isdir(p) and p not in sys.path:
            sys.path.insert(0, p)
    import concourse  # noqa: F401


_import_concourse()

import concourse.bacc as bacc  # noqa: E402
import concourse.tile as tile  # noqa: E402
from concourse import mybir  # noqa: E402
from concourse.bass_utils import run_bass_kernel_spmd  # noqa: E402

B = 512          # batch (pair-grid side)
D = 128          # input dim per tensor
H = 256          # hidden dim
NCORES = 8
RB = B // NCORES  # 64 y-rows per core
F32 = mybir.dt.float32

# float32r: fp32 bits in memory, single-pass reduced-precision multiply on
# the PE at 1 cycle/row (vs 4 for full fp32). Flip to mybir.dt.float32 for
# bit-accurate matmuls.
MM_DT = mybir.dt.float32r


def _src(ap):
    """Bitcast a DRAM fp32 AP to the matmul dtype for DMA into typed tiles.

    Walrus requires every tensor consumed by an FP32r matmul to be produced
    as float32r (DMA passthrough of fp32 bits is fine — same bit layout),
    so weight/activation tiles are allocated with dtype MM_DT and their
    producers write that dtype directly.
    """
    if MM_DT == F32:
        return ap
    return ap.bitcast(MM_DT)


def _emit(tc, nc, d, out_d):
    AF = mybir.ActivationFunctionType
    OP = mybir.AluOpType
    from contextlib import ExitStack

    with ExitStack() as ctx:
        const = ctx.enter_context(tc.tile_pool(name="const", bufs=1))
        hpool = ctx.enter_context(tc.tile_pool(name="h", bufs=2))
        psum = ctx.enter_context(tc.tile_pool(name="psum", bufs=1, space="PSUM"))

        def load(name, shape, src_ap=None, dt=F32):
            t = const.tile(list(shape), dt, tag=name, name=name + "_s")
            src = src_ap if src_ap is not None else d[name][:]
            if dt != F32:
                src = src.bitcast(dt)
            nc.sync.dma_start(out=t[:], in_=src)
            return t

        xT = load("xT", (D, B))
        yT = load("yT", (D, RB))
        w1t = load("W1t", (D, H))
        w1b = load("W1b", (D, H))
        w2 = [load(f"W2_{k}", (128, H), d["W2"][k * 128:(k + 1) * 128, :], MM_DT) for k in range(2)]
        w3 = [load(f"W3_{k}", (128, H), d["W3"][k * 128:(k + 1) * 128, :], MM_DT) for k in range(2)]
        w4 = [load(f"W4_{k}", (128, 1), d["W4"][k * 128:(k + 1) * 128, :], MM_DT) for k in range(2)]
        b1c = [load(f"b1_{k}", (128, 1), d["b1"][k * 128:(k + 1) * 128, :]) for k in range(2)]
        b2c = [load(f"b2_{k}", (128, 1), d["b2"][k * 128:(k + 1) * 128, :]) for k in range(2)]
        b3c = [load(f"b3_{k}", (128, 1), d["b3"][k * 128:(k + 1) * 128, :]) for k in range(2)]
        b4r = load("b4r", (RB, 1))

        # ---- preamble: xa[oc] = (x @ W1top)^T + b1 (bias folded here),
        #                yb[oc] = (y_slice @ W1bot)^T
        # Preamble psum tiles borrow the main-loop l2/l3 tags (no extra banks).
        xa = []
        yb = []
        for oc in range(2):
            ms = slice(oc * 128, (oc + 1) * 128)
            pxa = psum.tile([128, B], F32, tag=f"l2_{oc}", name=f"pxa{oc}", bufs=2)
            nc.tensor.matmul(pxa[:], lhsT=w1t[:, ms], rhs=xT[:], start=True, stop=True)
            xat = const.tile([128, B], F32, tag=f"xa{oc}", name=f"xa{oc}")
            nc.vector.tensor_scalar(xat[:], pxa[:], b1c[oc][:, 0:1], None, OP.add)
            xa.append(xat)
        for oc in range(2):
            ms = slice(oc * 128, (oc + 1) * 128)
            pyb = psum.tile([128, RB], F32, tag=f"l3_{oc}", name=f"pyb{oc}", bufs=1)
            nc.tensor.matmul(pyb[:], lhsT=w1b[:, ms], rhs=yT[:], start=True, stop=True)
            ybt = const.tile([128, RB], F32, tag=f"yb{oc}", name=f"yb{oc}")
            nc.vector.tensor_copy(ybt[:], pyb[:])
            yb.append(ybt)

        score = const.tile([RB, B], F32, tag="score", name="score")

        # Software-pipelined emission. Per step t the PE instruction stream is
        #   L2MM(t) | L4MM(t-2) | L3MM(t-1)
        # so every PE instruction only depends on epilogue work issued in a
        # PREVIOUS step (a full step of slack) — the per-chunk serial chain
        # L2MM->L2epi->L3MM->L3epi->L4MM never stalls the PE.
        # Engine split per step: ACT: 2x L2epi + exp(p4); DVE: 2x L1 + 2x L3epi.
        # PSUM banks: l2_0/l2_1 bufs=2 (4) + l3_0/l3_1 bufs=1 (2) + l4 bufs=2
        # (2) = 8 exactly.
        h1s, h2s, h3s, p4s = {}, {}, {}, {}

        def emit_l1(i):
            for oc in range(2):
                t = hpool.tile([128, B], MM_DT, tag=f"h1_{oc}", name=f"h1_{oc}_{i}")
                nc.vector.tensor_scalar(
                    t[:], xa[oc][:], yb[oc][:, i:i + 1], 0.0, OP.add, OP.max
                )
                h1s[i, oc] = t

        def emit_l2mm(i):
            for mc in range(2):
                ms = slice(mc * 128, (mc + 1) * 128)
                p = psum.tile([128, B], F32, tag=f"l2_{mc}", name=f"p2_{mc}_{i}", bufs=2)
                for kc in range(2):
                    nc.tensor.matmul(
                        p[:], lhsT=w2[kc][:, ms], rhs=h1s[i, kc][:],
                        start=(kc == 0), stop=(kc == 1),
                    )
                h2s[i, mc] = p  # psum handle; epi converts to SBUF below

        def emit_l2epi(i):
            for mc in range(2):
                p = h2s[i, mc]
                t = hpool.tile([128, B], MM_DT, tag=f"h2_{mc}", name=f"h2_{mc}_{i}")
                nc.scalar.activation(t[:], p[:], AF.Relu, bias=b2c[mc][:, 0:1])
                h2s[i, mc] = t

        def emit_l3mm(i):
            for mc in range(2):
                ms = slice(mc * 128, (mc + 1) * 128)
                p = psum.tile([128, B], F32, tag=f"l3_{mc}", name=f"p3_{mc}_{i}", bufs=1)
                for kc in range(2):
                    nc.tensor.matmul(
                        p[:], lhsT=w3[kc][:, ms], rhs=h2s[i, kc][:],
                        start=(kc == 0), stop=(kc == 1),
                    )
                h3s[i, mc] = p

        def emit_l3epi(i):
            for mc in range(2):
                p = h3s[i, mc]
                t = hpool.tile([128, B], MM_DT, tag=f"h3_{mc}", name=f"h3_{mc}_{i}")
                nc.vector.tensor_scalar(t[:], p[:], b3c[mc][:, 0:1], 0.0, OP.add, OP.max)
                h3s[i, mc] = t
            del h2s[i, 0], h2s[i, 1]

        def emit_l4mm(i):
            p4 = psum.tile([1, B], F32, tag="l4", name=f"p4_{i}", bufs=2)
            for kc in range(2):
                nc.tensor.matmul(
                    p4[:], lhsT=w4[kc][:], rhs=h3s[i, kc][:],
                    start=(kc == 0), stop=(kc == 1),
                )
            p4s[i] = p4
            del h3s[i, 0], h3s[i, 1]

        def emit_drain(i):
            # exp(score + b4) per row; the batched tail only needs Ln then.
            # Engines can only write partition bases {0,32,64,96}, so stage
            # through a base-0 tile and DMA into row i (DMA targets any
            # partition).
            s4 = hpool.tile([1, B], F32, tag="s4", name=f"s4_{i}")
            nc.scalar.activation(s4[:], p4s.pop(i)[:], AF.Exp, bias=b4r[0:1, 0:1])
            nc.sync.dma_start(out=score[i:i + 1, :], in_=s4[:])

        emit_l1(0)
        for t in range(RB + 2):
            if t + 1 < RB:
                emit_l1(t + 1)
            if t < RB:
                emit_l2mm(t)
                emit_l2epi(t)
            if t >= 2:
                emit_l4mm(t - 2)
                emit_drain(t - 2)
            if t >= 1 and t - 1 < RB:
                emit_l3mm(t - 1)
                emit_l3epi(t - 1)

        # ---- tail: softplus = ln(1 + e). Relu/Exp/Ln all live in the
        # natural_log_exp_and_others ACT table -> single table load.
        fin = const.tile([RB, B], F32, tag="fin", name="fin")
        nc.scalar.activation(fin[:], score[:], AF.Ln, bias=1.0)

        nc.sync.dma_start(out=out_d[:], in_=fin[:])


def _build_program():
    nc = bacc.Bacc("TRN2", target_bir_lowering=False, debug=False, enable_asserts=False)
    d = {}
    for name, shape in [
        ("xT", (D, B)), ("yT", (D, RB)),
        ("W1t", (D, H)), ("W1b", (D, H)),
        ("W2", (H, H)), ("W3", (H, H)), ("W4", (H, 1)),
        ("b1", (H, 1)), ("b2", (H, 1)), ("b3", (H, 1)), ("b4r", (RB, 1)),
    ]:
        d[name] = nc.dram_tensor(name, list(shape), F32, kind="ExternalInput").ap()
    out_d = nc.dram_tensor("out", [RB, B], F32, kind="ExternalOutput").ap()
    with tile.TileContext(nc) as tc:
        _emit(tc, nc, d, out_d)
    nc.compile()
    return nc


_PROGRAM = None


def _get_program():
    global _PROGRAM
    if _PROGRAM is None:
        _PROGRAM = _build_program()
    return _PROGRAM


def _make_in_maps(x, y, W1, b1, W2, b2, W3, b3, W4, b4):
    f = np.float32
    xT = np.ascontiguousarray(x.T, dtype=f)
    shared = {
        "xT": xT,
        "W1t": np.ascontiguousarray(W1[:D], dtype=f),
        "W1b": np.ascontiguousarray(W1[D:], dtype=f),
        "W2": np.ascontiguousarray(W2, dtype=f),
        "W3": np.ascontiguousarray(W3, dtype=f),
        "W4": np.ascontiguousarray(W4.reshape(H, 1), dtype=f),
        "b1": np.ascontiguousarray(b1.reshape(H, 1), dtype=f),
        "b2": np.ascontiguousarray(b2.reshape(H, 1), dtype=f),
        "b3": np.ascontiguousarray(b3.reshape(H, 1), dtype=f),
        "b4r": np.full((RB, 1), np.asarray(b4, dtype=f).reshape(-1)[0], dtype=f),
    }
    in_maps = []
    for c in range(NCORES):
        m = dict(shared)
        m["yT"] = np.ascontiguousarray(y[c * RB:(c + 1) * RB].T, dtype=f)
        in_maps.append(m)
    return in_maps


def _run(inputs, trace=False, trace_cores=None):
    nc = _get_program()
    in_maps = _make_in_maps(**inputs)
    res = run_bass_kernel_spmd(
        nc, in_maps, list(range(NCORES)), trace=trace, trace_cores=trace_cores,
    )
    V = np.concatenate([res.results[c]["out"] for c in range(NCORES)], axis=0)
    out = np.ascontiguousarray(V.T, dtype=np.float32)
    return out, res


def kernel(**inputs):
    out, _ = _run(inputs, trace=False)
    return out
